# revision 8
# baseline (speedup 1.0000x reference)
"""Trainium2 Bass kernel for 3D Catmull-Rom cubic spline grid interpolation.

Problem: grid (2, 64, 64, 64) f32, u (1_000_000, 3) in [0,1]^3 -> out (1_000_000, 2).

Sharding: data-parallel over query points (N axis) across 8 NeuronCores, per the
sharding hint. Host-side prep per shard: computes integer cell indices (floor),
assembles the per-point 4x4x4 control-point neighborhood into a dense
[128, n] operand (c,a,b,j tap-major), and transposes coords. The device kernel
computes, for every point, the Catmull-Rom weight tensor for all 64 taps x 2
channels and contracts it against the neighborhood values:

  device per point:
    t3    = u*63 - floor(u*63)                (DVE)
    ln3   = Ln(max(t3, eps))                  (ACT)
    S     = Emap^T @ ln3   [64 monomials]     (PE)   S[(i,j,k)] = i*ln tz + j*ln ty + k*ln tx
    mono  = Exp(S)                            (ACT)  mono = tz^i * ty^j * tx^k
    W     = C^T @ mono     [128 taps]         (PE)   W[(c,a,b,jx)] = wz_a(tz)*wy_b(ty)*wx_jx(tx)
    M     = vals * W                          (DVE)
    out   = Ones^T @ M     [2 channels]       (PE)   sum over the 64 taps per channel

All weight-polynomial evaluation and the full interpolation contraction
(~260 FLOP/point) run on device; the host contributes data layout only
(shard, floor/clip indices, neighborhood assembly, transposes).
"""

import numpy as np
from contextlib import ExitStack

import sys

sys.path.insert(0, "/opt/trn_rl_repo")

import concourse.bass as bass
import concourse.tile as tile
from concourse import bacc
from concourse import mybir
from concourse.bass_utils import run_bass_kernel_spmd

# ---------------------------------------------------------------- constants
N_POINTS = 1_000_000
N_CORES = 8
CHUNK = 512
N_PER_CORE = N_POINTS // N_CORES            # 125000
N_PAD = ((N_PER_CORE + CHUNK - 1) // CHUNK) * CHUNK   # 125440
N_CHUNKS = N_PAD // CHUNK                   # 245
RES = 64
EPS = 1e-9

CATMULL_ROM_MATRIX = 0.5 * np.array(
    [[0.0, 2.0, 0.0, 0.0],
     [-1.0, 0.0, 1.0, 0.0],
     [2.0, -5.0, 4.0, -1.0],
     [-1.0, 3.0, -3.0, 1.0]], dtype=np.float32)


def _host_constants():
    M = CATMULL_ROM_MATRIX.astype(np.float64)
    # Emap [3, 64]: monomial m=(i,j,k) -> exponents per axis (z,y,x order).
    emap = np.zeros((3, 64), dtype=np.float32)
    # C [64 monomials, 128 taps]: taps p = c*64 + a*16 + b*4 + j.
    cmat = np.zeros((64, 128), dtype=np.float32)
    for i in range(4):
        for j in range(4):
            for k in range(4):
                m = i * 16 + j * 4 + k
                emap[0, m] = i
                emap[1, m] = j
                emap[2, m] = k
                for a in range(4):
                    for b in range(4):
                        for jx in range(4):
                            w = M[i, a] * M[j, b] * M[k, jx]
                            p = a * 16 + b * 4 + jx
                            cmat[m, p] = w
                            cmat[m, 64 + p] = w
    ones2 = np.zeros((128, 2), dtype=np.float32)
    ones2[:64, 0] = 1.0
    ones2[64:, 1] = 1.0
    return emap, cmat, ones2


def _build_bass():
    nc = bacc.Bacc("TRN2", target_bir_lowering=False, debug=False,
                   num_devices=N_CORES)
    f32 = mybir.dt.float32
    vals = nc.dram_tensor("vals", [128, N_PAD], f32, kind="ExternalInput").ap()
    coordT = nc.dram_tensor("coordT", [3, 2, N_PAD], f32, kind="ExternalInput").ap()
    emap = nc.dram_tensor("emap", [3, 64], f32, kind="ExternalInput").ap()
    cmat = nc.dram_tensor("cmat", [64, 128], f32, kind="ExternalInput").ap()
    ones2 = nc.dram_tensor("ones2", [128, 2], f32, kind="ExternalInput").ap()
    outT = nc.dram_tensor("outT", [2, N_PAD], f32, kind="ExternalOutput").ap()

    with tile.TileContext(nc) as tc, ExitStack() as ctx:
        consts = ctx.enter_context(tc.tile_pool(name="consts", bufs=1))
        inp = ctx.enter_context(tc.tile_pool(name="inp", bufs=4))
        small = ctx.enter_context(tc.tile_pool(name="small", bufs=3))
        mid = ctx.enter_context(tc.tile_pool(name="mid", bufs=3))
        outp = ctx.enter_context(tc.tile_pool(name="outp", bufs=3))
        psS_pool = ctx.enter_context(tc.tile_pool(name="psS", bufs=2, space="PSUM"))
        psW_pool = ctx.enter_context(tc.tile_pool(name="psW", bufs=2, space="PSUM"))
        psO_pool = ctx.enter_context(tc.tile_pool(name="psO", bufs=2, space="PSUM"))

        emap_sb = consts.tile([3, 64], f32, tag="emap")
        nc.sync.dma_start(out=emap_sb[:], in_=emap[:, :])
        cmat_sb = consts.tile([64, 128], f32, tag="cmat")
        nc.sync.dma_start(out=cmat_sb[:], in_=cmat[:, :])
        ones2_sb = consts.tile([128, 2], f32, tag="ones2")
        nc.sync.dma_start(out=ones2_sb[:], in_=ones2[:, :])

        for i in range(N_CHUNKS):
            sl = slice(i * CHUNK, (i + 1) * CHUNK)
            c6 = small.tile([3, 2, CHUNK], f32, tag="c6")
            nc.sync.dma_start(out=c6[:], in_=coordT[:, :, sl])
            v = inp.tile([128, CHUNK], f32, tag="v")
            nc.sync.dma_start(out=v[:], in_=vals[:, sl])

            # t = u*63 - icell ; clamp to eps so Ln is finite
            p3 = small.tile([3, CHUNK], f32, tag="p3")
            nc.vector.tensor_scalar(
                out=p3[:], in0=c6[:, 0, :], scalar1=63.0, scalar2=None,
                op0=mybir.AluOpType.mult)
            t3 = small.tile([3, CHUNK], f32, tag="t3")
            nc.vector.tensor_sub(t3[:], p3[:], c6[:, 1, :])
            t3c = small.tile([3, CHUNK], f32, tag="t3c")
            nc.vector.tensor_scalar(
                out=t3c[:], in0=t3[:], scalar1=EPS, scalar2=None,
                op0=mybir.AluOpType.max)
            ln3 = small.tile([3, CHUNK], f32, tag="ln3")
            nc.scalar.activation(ln3[:], t3c[:],
                                 mybir.ActivationFunctionType.Ln)

            # S[64, n] = Emap^T @ ln3 ; mono = exp(S)
            psS = psS_pool.tile([64, CHUNK], f32, tag="psS")
            nc.tensor.matmul(psS[:], emap_sb[:], ln3[:], start=True, stop=True)
            mono = mid.tile([64, CHUNK], f32, tag="mono")
            nc.scalar.activation(mono[:], psS[:],
                                 mybir.ActivationFunctionType.Exp)

            # W[128, n] = C^T @ mono ; M = vals * W
            psW = psW_pool.tile([128, CHUNK], f32, tag="psW")
            nc.tensor.matmul(psW[:], cmat_sb[:], mono[:], start=True, stop=True)
            m = inp.tile([128, CHUNK], f32, tag="m")
            nc.vector.tensor_mul(m[:], v[:], psW[:])

            # out[2, n] = Ones^T @ M
            psO = psO_pool.tile([2, CHUNK], f32, tag="psO")
            nc.tensor.matmul(psO[:], ones2_sb[:], m[:], start=True, stop=True)
            ob = outp.tile([2, CHUNK], f32, tag="ob")
            nc.scalar.copy(ob[:], psO[:])
            nc.sync.dma_start(out=outT[:, sl], in_=ob[:])

    nc.compile()
    return nc


def kernel(grid: np.ndarray, u: np.ndarray) -> np.ndarray:
    grid = np.asarray(grid, dtype=np.float32)
    u = np.asarray(u, dtype=np.float32)
    n = u.shape[0]
    assert n == N_POINTS and grid.shape == (2, RES, RES, RES)

    emap, cmat, ones2 = _host_constants()

    pos = u * np.float32(RES - 1)              # f32, matches reference
    icell = np.clip(np.floor(pos), 0, RES - 2).astype(np.int32)   # (N, 3)
    offs = np.arange(-1, 3, dtype=np.int32)
    # ctrl indices per axis, clipped — (N, 3, 4)
    ctrl = np.clip(icell[:, :, None] + offs[None, None, :], 0, RES - 1)

    in_maps = []
    for c in range(N_CORES):
        s = slice(c * N_PER_CORE, (c + 1) * N_PER_CORE)
        cz = ctrl[s, 0]                        # (n, 4)
        cy = ctrl[s, 1]
        cx = ctrl[s, 2]
        # vals[p = ch*64 + a*16 + b*4 + j, n]
        g = grid[:, cz[:, :, None, None], cy[:, None, :, None], cx[:, None, None, :]]
        g = np.transpose(g, (0, 2, 3, 4, 1)).reshape(128, N_PER_CORE)
        vals = np.zeros((128, N_PAD), dtype=np.float32)
        vals[:, :N_PER_CORE] = np.ascontiguousarray(g)
        coordT = np.zeros((3, 2, N_PAD), dtype=np.float32)
        coordT[:, 0, :N_PER_CORE] = u[s].T
        coordT[:, 1, :N_PER_CORE] = icell[s].T.astype(np.float32)
        in_maps.append({"vals": vals, "coordT": coordT,
                        "emap": emap, "cmat": cmat, "ones2": ones2})

    nc = _build_bass()
    res = run_bass_kernel_spmd(nc, in_maps, list(range(N_CORES)))

    out = np.empty((n, 2), dtype=np.float32)
    for c in range(N_CORES):
        r = res.results[c]
        o = r["outT"] if "outT" in r else r[[k for k in r if "outT" in k][0]]
        out[c * N_PER_CORE:(c + 1) * N_PER_CORE, :] = o[:, :N_PER_CORE].T
    return out
